# revision 2
# baseline (speedup 1.0000x reference)
"""GNN message-passing layer on 8 Trainium2 NeuronCores.

Computes out[i] = relu(U @ (sum_{j: adj[j,i]>0} x[j]) / deg_i) where
deg_i = sum_j adj[j,i], for a dense binary adjacency matrix.

Sharding: columns i of adj (target nodes) are row-sharded across the 8
cores (core g owns i in [g*2048, (g+1)*2048)); x and U are replicated.

Each core streams its 32MB fp8 adjacency shard once through the tensor
engine (the binary adjacency is exact in fp8e4, so the host stages it as
raw fp8 bytes and the DMA is a plain HWDGE copy — no cast anywhere):

  stage 1:  one fp8 DoubleRow matmul pass. The PE contracts TWO 128-row
            j-tiles per instruction at 2 moving cols/cycle (4x the bf16
            rate). The full 128 stationary columns hold a hi/lo split of
            x so one pass gives ~fp16-grade accuracy:
              cols   0..63 : x_hi[d]     (fp8 of x, all 64 dims)
              col       64 : 1.0         (accumulates deg_i)
              cols 65..127 : x_lo[d]*16  (fp8 of (x - fp8(x))*16, d<63)
            PSUM accumulates [128, 2048] fp32 over all 64 tile-pairs.
  stage 2:  out[i,:] = relu((agg_hi @ U^T + agg_lo @ U^T/16) * (1/deg_i))
            via ONE 128-row-contraction matmul per 128-node chunk with
            strided stationary + the hi/lo recombine (and deg-row cancel)
            folded into the moving tensor [U^T | deg-sel | U^T[0:63]/16].
            A 65th moving column drops each chunk's deg into PSUM col 64
            already chunk-transposed, so the 1/deg scale comes from a
            [128,1] DVE reciprocal — no DRAM round-trip to re-layout deg.
            The relu/scale is fused in one DVE tensor_scalar per chunk;
            the output store is bf16 (halves store bytes; adds ~1.7e-3
            rel err, total ~3e-3, far under the 2e-2 gate) and the host
            casts back to f32.

DMA queue use (measured on hardware, slope-timed over NEFF-internal
repetition): the per-core HBM controller sustains only ~350 GB/s
steady-state no matter how many DGE queues are active, and a SINGLE
queue walking DRAM sequentially is the most efficient transport —
column- or region-splitting the stream across the two HWDGE rings
and/or SWDGE always degraded sustained throughput (queue arbitration
breaks row-buffer locality). Fat descriptors (4-32KB via row-grouped
SBUF layouts) also measured no better than the plain 2KB-per-partition
layout. So: the whole adjacency stream rides the sync/SP HWDGE ring
sequentially, while the small x''/U^T loads and the bf16 output store
ride the scalar/ACT ring where they perturb the bulk stream least.

Dim 63 of x has no lo half (only 127 stationary cols remain after deg);
its error is ~fp8-grade but affects 1/64 dims -> ~2.5e-3 overall rel
err, within the 2e-2 gate.
"""

import sys

sys.path.insert(0, "/opt/trn_rl_repo")

import numpy as np
from contextlib import ExitStack

import concourse.bass as bass
import concourse.bacc as bacc
import concourse.mybir as mybir
import concourse.tile as tile
from concourse.bass_utils import run_bass_kernel_spmd

N_CORES = 8
P = 128

N_NODES = 16384
D = 64
SHARD = N_NODES // N_CORES  # 2048

TPD2 = 4  # j-tile PAIRS per slab -> slab = [tpd2*2*128 rows, SHARD]


def build_nc(
    n=N_NODES,
    d=D,
    shard=SHARD,
    tpd2=TPD2,
    reps=1,
    bufs=6,
    splits=((("sync"), 16),),
    xq="scalar",
    outq="scalar",
    out_bf16=True,
    const_bufs=1,
):
    """reps > 1 repeats the whole kernel body inside one NEFF — used only to
    measure per-invocation time as a slope (the axon RPC overhead per launch
    is ~90ms, far above the kernel time)."""
    njt2 = n // (2 * P)
    nslab = njt2 // tpd2
    cg = shard // P
    f32 = mybir.dt.float32
    f8 = mybir.dt.float8e4
    out_dt = mybir.dt.bfloat16 if out_bf16 else f32

    assert sum(ns for _, ns in splits) == nslab, splits

    # Bacc (not raw Bass): its compile() splits multi-semaphore waits into
    # event semaphores — TRN2 ISA instructions fit only one wait.
    nc = bacc.Bacc()
    a_sh = nc.declare_dram_parameter("a_sh", [n, shard], f8, isOutput=False)
    xp = nc.declare_dram_parameter("xp", [P, njt2 * 2 * P], f8, isOutput=False)
    ut = nc.declare_dram_parameter("ut", [P, d + 1], f32, isOutput=False)
    out = nc.declare_dram_parameter("out_sh", [shard, d], out_dt, isOutput=True)

    # issue/consume order: round-robin across queues, each walking its own
    # contiguous slab region (PSUM accumulation commutes, so the PE can
    # consume slabs in arrival order)
    order = []  # (queue, slab_idx)
    bounds = []
    acc = 0
    for q, ns in splits:
        bounds.append((q, acc, acc + ns))
        acc += ns
    cursors = [[q, s0, s1, s0] for q, s0, s1 in bounds]
    while True:
        progressed = False
        for cur in cursors:
            if cur[3] < cur[2]:
                order.append((cur[0], cur[3]))
                cur[3] += 1
                progressed = True
        if not progressed:
            break

    with tile.TileContext(nc) as tc:
        with ExitStack() as ctx:
            constp = ctx.enter_context(tc.tile_pool(name="const", bufs=const_bufs))
            utp = ctx.enter_context(tc.tile_pool(name="utp", bufs=1))
            apool = ctx.enter_context(tc.tile_pool(name="adj", bufs=bufs))
            pspool = ctx.enter_context(tc.tile_pool(name="psum1", bufs=1, space="PSUM"))
            ps2pool = ctx.enter_context(tc.tile_pool(name="psum2", bufs=2, space="PSUM"))
            postp = ctx.enter_context(tc.tile_pool(name="post", bufs=1))

            first_t2 = order[0][1] * tpd2
            last_t2 = order[-1][1] * tpd2 + (tpd2 - 1)

            def emit_body():
                # stationary: partition p holds x''[(2*t2+s)*128+p, c] at
                # free offset (t2, s, c); c = [x_hi | 1 | x_lo*16] layout.
                x_sb = constp.tile([P, njt2, 2, P], f8, tag="x")
                getattr(nc, xq).dma_start(
                    x_sb[:], xp[:, :].rearrange("p (t s c) -> p t s c", s=2, c=P)
                )
                ut_sb = utp.tile([P, d + 1], f32, tag="ut")
                getattr(nc, xq).dma_start(ut_sb[:], ut[:, :])

                # stage 1: aggT = x''^T @ a over all j-tile pairs (DoubleRow)
                ps_agg = pspool.tile([P, shard], f32, tag="agg")
                for q, sl in order:
                    rows = a_sh[sl * tpd2 * 2 * P : (sl + 1) * tpd2 * 2 * P, :]
                    a_t = apool.tile([P, tpd2, 2, shard], f8, tag="a")
                    getattr(nc, q).dma_start(
                        a_t[:], rows.rearrange("(t s p) c -> p t s c", p=P, s=2)
                    )
                    for tt in range(tpd2):
                        t2 = sl * tpd2 + tt
                        # 256-wide out chunks: the DoubleRow moving AP is
                        # [128, 2, w] = 2w free elements and the PE caps the
                        # moving free dim at 512 (wider hangs the engine).
                        # start=True zeroes the full 2KB PSUM bank, so only
                        # the bank-aligned chunk carries it; the odd chunk
                        # inherits the bank's pending-zero.
                        for c0 in range(0, shard, 256):
                            w = min(256, shard - c0)
                            nc.tensor.matmul(
                                ps_agg[:, c0 : c0 + w],
                                x_sb[:, t2, :, :],
                                a_t[:, tt, :, c0 : c0 + w],
                                start=(t2 == first_t2 and c0 % 512 == 0),
                                stop=(t2 == last_t2),
                                perf_mode=mybir.MatmulPerfMode.DoubleRow,
                                skip_group_check=(c0 % 512 != 0),
                            )

                # epilogue: rows 0..63 = agg_hi, row 64 = deg,
                # rows 65..127 = agg_lo*16.  Stays on DVE: ACT's sequencer
                # owns the scalar-ring aux DMAs.
                agg_sb = postp.tile([P, shard], f32, tag="aggsb")
                nc.vector.tensor_copy(agg_sb[:], ps_agg[:])

                out_sb = postp.tile([P, cg * d], out_dt, tag="out")
                rec_t = postp.tile([P, cg], f32, tag="rec")
                # i = m*cg + cpos: strided stationary picks every cg-th column
                agg_all = agg_sb[:, :].rearrange("dd (m c) -> dd c m", c=cg)
                for cpos in range(cg):
                    ps2 = ps2pool.tile([P, d + 1], f32, tag="p2")
                    nc.tensor.matmul(
                        ps2[:],
                        agg_all[:, cpos, :],
                        ut_sb[:, :],
                        start=True,
                        stop=True,
                    )
                    nc.vector.reciprocal(
                        rec_t[:, cpos : cpos + 1], ps2[:, d : d + 1]
                    )
                    # out = max(ps2 * (1/deg), 0) fused in one DVE op
                    nc.vector.tensor_scalar(
                        out_sb[:, cpos * d : (cpos + 1) * d],
                        ps2[:, 0:d],
                        rec_t[:, cpos : cpos + 1],
                        0.0,
                        mybir.AluOpType.mult,
                        mybir.AluOpType.max,
                    )
                getattr(nc, outq).dma_start(
                    out[:, :].rearrange("(m c) dd -> m (c dd)", c=cg), out_sb[:]
                )

            for _rep in range(reps):
                emit_body()
    nc.compile()
    return nc


def make_host_inputs(x, adj_mat, U, n=N_NODES, d=D, shard=SHARD, n_cores=N_CORES):
    """Build per-core input maps from the full problem inputs (dtype/layout
    transforms only — all arithmetic on the data happens on-device)."""
    np8 = mybir.dt.np(mybir.dt.float8e4)
    njt2 = n // (2 * P)

    x = np.asarray(x, dtype=np.float32)
    U = np.asarray(U, dtype=np.float32)

    x_hi8 = x.astype(np8)
    x_lo8 = ((x - x_hi8.astype(np.float32)) * 16.0).astype(np8)
    xpp = np.zeros((n, P), np8)
    xpp[:, 0:d] = x_hi8
    xpp[:, d] = np.ones((), np.float32).astype(np8)
    xpp[:, d + 1 : 2 * d] = x_lo8[:, 0 : d - 1]
    xp_t = np.ascontiguousarray(
        xpp.reshape(njt2, 2, P, P).transpose(2, 0, 1, 3).reshape(P, njt2 * 2 * P)
    )

    ut_full = np.zeros((P, d + 1), np.float32)
    ut_full[0:d, 0:d] = U.T
    ut_full[d + 1 : 2 * d, 0:d] = U.T[0 : d - 1] / 16.0  # row d: 0 cancels deg
    ut_full[d, d] = 1.0  # deg-selector column -> PSUM col d holds deg_i

    adj8 = np.asarray(adj_mat).astype(np8)  # binary values: exact in fp8
    in_maps = []
    for g in range(n_cores):
        a_sh = np.ascontiguousarray(adj8[:, g * shard : (g + 1) * shard])
        in_maps.append({"a_sh": a_sh, "xp": xp_t, "ut": ut_full})
    return in_maps


_NC_CACHE = {}


def get_nc(**kw):
    key = repr(sorted(kw.items()))
    if key not in _NC_CACHE:
        _NC_CACHE[key] = build_nc(**kw)
    return _NC_CACHE[key]


def kernel(x, adj_mat, U, **run_kw):
    """Full inputs in, full output out. Shards across 8 NeuronCores."""
    in_maps = make_host_inputs(x, adj_mat, U)
    nc = get_nc()
    res = run_bass_kernel_spmd(nc, in_maps, core_ids=list(range(N_CORES)), **run_kw)
    out = np.concatenate(
        [np.asarray(res.results[g]["out_sh"]).astype(np.float32) for g in range(N_CORES)],
        axis=0,
    )
    if run_kw:
        kernel.last_result = res
    return out


# revision 5
# speedup vs baseline: 1.0160x; 1.0160x over previous
"""GNN message-passing layer on 8 Trainium2 NeuronCores.

Computes out[i] = relu(U @ (sum_{j: adj[j,i]>0} x[j]) / deg_i) where
deg_i = sum_j adj[j,i], for a dense binary adjacency matrix.

Sharding: columns i of adj (target nodes) are row-sharded across the 8
cores (core g owns i in [g*2048, (g+1)*2048)); x and U are replicated.

Each core streams its 32MB fp8 adjacency shard once through the tensor
engine (the binary adjacency is exact in fp8e4, so the host stages it as
raw fp8 bytes and the DMA is a plain HWDGE copy — no cast anywhere):

  stage 1:  one fp8 DoubleRow matmul pass. The PE contracts TWO 128-row
            j-tiles per instruction at 2 moving cols/cycle (4x the bf16
            rate). The full 128 stationary columns hold a hi/lo split of
            x so one pass gives ~fp16-grade accuracy:
              cols   0..63 : x_hi[d]     (fp8 of x, all 64 dims)
              col       64 : 1.0         (accumulates deg_i)
              cols 65..127 : x_lo[d]*16  (fp8 of (x - fp8(x))*16, d<63)
            PSUM accumulates [128, 2048] fp32 over all 64 tile-pairs.
  stage 2:  out[i,:] = relu((agg_hi @ U^T + agg_lo @ U^T/16) * (1/deg_i))
            via ONE 128-row-contraction matmul per 128-node chunk with
            strided stationary + the hi/lo recombine (and deg-row cancel)
            folded into the moving tensor [U^T | deg-sel | U^T[0:63]/16].
            A 65th moving column drops each chunk's deg into PSUM col 64
            already chunk-transposed, so the 1/deg scale comes from a
            [128,1] DVE reciprocal — no DRAM round-trip to re-layout deg.
            The relu/scale is fused in one DVE tensor_scalar per chunk;
            the output store is bf16 (halves store bytes; adds ~1.7e-3
            rel err, total ~3e-3, far under the 2e-2 gate) and the host
            casts back to f32.

DMA queue use (measured on hardware, slope-timed over NEFF-internal
repetition): the per-core HBM controller sustains only ~350 GB/s
steady-state no matter how many DGE queues are active, and a SINGLE
queue walking DRAM sequentially is the most efficient transport —
column- or region-splitting the stream across the two HWDGE rings
and/or SWDGE always degraded sustained throughput (queue arbitration
breaks row-buffer locality). Fat descriptors (4-32KB via row-grouped
SBUF layouts) also measured no better than the plain 2KB-per-partition
layout. So: the whole adjacency stream rides the sync/SP HWDGE ring
sequentially, while the small x''/U^T loads and the bf16 output store
ride the scalar/ACT ring where they perturb the bulk stream least.

Dim 63 of x has no lo half (only 127 stationary cols remain after deg);
its error is ~fp8-grade but affects 1/64 dims -> ~2.5e-3 overall rel
err, within the 2e-2 gate.
"""

import sys

sys.path.insert(0, "/opt/trn_rl_repo")

import numpy as np
from contextlib import ExitStack

import concourse.bass as bass
import concourse.bacc as bacc
import concourse.mybir as mybir
import concourse.tile as tile
from concourse.bass_utils import run_bass_kernel_spmd

N_CORES = 8
P = 128

N_NODES = 16384
D = 64
SHARD = N_NODES // N_CORES  # 2048

TPD2 = 4  # j-tile PAIRS per slab -> slab = [tpd2*2*128 rows, SHARD]


def build_nc(
    n=N_NODES,
    d=D,
    shard=SHARD,
    tpd2=TPD2,
    reps=1,
    bufs=6,
    splits=((("sync"), 16),),
    xq="scalar",
    outq="scalar",
    out_bf16=True,
    const_bufs=1,
    early_reps=0,
    early_cols=(896, 1792),
):
    """reps > 1 repeats the whole kernel body inside one NEFF — used only to
    measure per-invocation time as a slope (the axon RPC overhead per launch
    is ~90ms, far above the kernel time)."""
    njt2 = n // (2 * P)
    nslab = njt2 // tpd2
    cg = shard // P
    f32 = mybir.dt.float32
    f8 = mybir.dt.float8e4
    out_dt = mybir.dt.bfloat16 if out_bf16 else f32

    assert sum(ns for _, ns in splits) == nslab, splits

    # Bacc (not raw Bass): its compile() splits multi-semaphore waits into
    # event semaphores — TRN2 ISA instructions fit only one wait.
    nc = bacc.Bacc()
    a_sh = nc.declare_dram_parameter("a_sh", [n, shard], f8, isOutput=False)
    xp = nc.declare_dram_parameter("xp", [P, njt2 * 2 * P], f8, isOutput=False)
    ut = nc.declare_dram_parameter("ut", [P, d + 1], f32, isOutput=False)
    out = nc.declare_dram_parameter("out_sh", [shard, d], out_dt, isOutput=True)

    # issue/consume order: round-robin across queues, each walking its own
    # contiguous slab region (PSUM accumulation commutes, so the PE can
    # consume slabs in arrival order)
    order = []  # (queue, slab_idx)
    bounds = []
    acc = 0
    for q, ns in splits:
        bounds.append((q, acc, acc + ns))
        acc += ns
    cursors = [[q, s0, s1, s0] for q, s0, s1 in bounds]
    while True:
        progressed = False
        for cur in cursors:
            if cur[3] < cur[2]:
                order.append((cur[0], cur[3]))
                cur[3] += 1
                progressed = True
        if not progressed:
            break

    with tile.TileContext(nc) as tc:
        with ExitStack() as ctx:
            constp = ctx.enter_context(tc.tile_pool(name="const", bufs=const_bufs))
            utp = ctx.enter_context(tc.tile_pool(name="utp", bufs=1))
            apool = ctx.enter_context(tc.tile_pool(name="adj", bufs=bufs))
            pspool = ctx.enter_context(tc.tile_pool(name="psum1", bufs=1, space="PSUM"))
            ps2pool = ctx.enter_context(tc.tile_pool(name="psum2", bufs=2, space="PSUM"))
            postp = ctx.enter_context(tc.tile_pool(name="post", bufs=1))

            first_t2 = order[0][1] * tpd2
            last_t2 = order[-1][1] * tpd2 + (tpd2 - 1)

            def emit_body(early=False):
                # stationary: partition p holds x''[(2*t2+s)*128+p, c] at
                # free offset (t2, s, c); c = [x_hi | 1 | x_lo*16] layout.
                x_sb = constp.tile([P, njt2, 2, P], f8, tag="x")
                getattr(nc, xq).dma_start(
                    x_sb[:], xp[:, :].rearrange("p (t s c) -> p t s c", s=2, c=P)
                )
                ut_sb = utp.tile([P, d + 1], f32, tag="ut")
                getattr(nc, xq).dma_start(ut_sb[:], ut[:, :])

                # stage 1: aggT = x''^T @ a over all j-tile pairs (DoubleRow)
                ps_agg = pspool.tile([P, shard], f32, tag="agg")
                for q, sl in order:
                    rows = a_sh[sl * tpd2 * 2 * P : (sl + 1) * tpd2 * 2 * P, :]
                    a_t = apool.tile([P, tpd2, 2, shard], f8, tag="a")
                    src = rows.rearrange("(t s p) c -> p t s c", p=P, s=2)
                    if early:
                        # burst-phase transport: all three DGE queues split
                        # each slab's columns — the HBM controller exceeds
                        # 450 GB/s for the first ~2-3ms of an execution, and
                        # multi-queue is the only way to pull that rate.
                        b0, b1 = early_cols
                        nc.sync.dma_start(a_t[:, :, :, 0:b0], src[:, :, :, 0:b0])
                        nc.scalar.dma_start(
                            a_t[:, :, :, b0:b1], src[:, :, :, b0:b1]
                        )
                        nc.gpsimd.dma_start(a_t[:, :, :, b1:], src[:, :, :, b1:])
                    else:
                        getattr(nc, q).dma_start(a_t[:], src)
                    for tt in range(tpd2):
                        t2 = sl * tpd2 + tt
                        # 256-wide out chunks: the DoubleRow moving AP is
                        # [128, 2, w] = 2w free elements and the PE caps the
                        # moving free dim at 512 (wider hangs the engine).
                        # start=True zeroes the full 2KB PSUM bank, so only
                        # the bank-aligned chunk carries it; the odd chunk
                        # inherits the bank's pending-zero.
                        for c0 in range(0, shard, 256):
                            w = min(256, shard - c0)
                            nc.tensor.matmul(
                                ps_agg[:, c0 : c0 + w],
                                x_sb[:, t2, :, :],
                                a_t[:, tt, :, c0 : c0 + w],
                                start=(t2 == first_t2 and c0 % 512 == 0),
                                stop=(t2 == last_t2),
                                perf_mode=mybir.MatmulPerfMode.DoubleRow,
                                skip_group_check=(c0 % 512 != 0),
                            )

                # epilogue: rows 0..63 = agg_hi, row 64 = deg,
                # rows 65..127 = agg_lo*16.  Stays on DVE: ACT's sequencer
                # owns the scalar-ring aux DMAs.
                agg_sb = postp.tile([P, shard], f32, tag="aggsb")
                nc.vector.tensor_copy(agg_sb[:], ps_agg[:])

                out_sb = postp.tile([P, cg * d], out_dt, tag="out")
                rec_t = postp.tile([P, cg], f32, tag="rec")
                # i = m*cg + cpos: strided stationary picks every cg-th column
                agg_all = agg_sb[:, :].rearrange("dd (m c) -> dd c m", c=cg)
                for cpos in range(cg):
                    ps2 = ps2pool.tile([P, d + 1], f32, tag="p2")
                    nc.tensor.matmul(
                        ps2[:],
                        agg_all[:, cpos, :],
                        ut_sb[:, :],
                        start=True,
                        stop=True,
                    )
                    nc.vector.reciprocal(
                        rec_t[:, cpos : cpos + 1], ps2[:, d : d + 1]
                    )
                    # out = max(ps2 * (1/deg), 0) fused in one DVE op
                    nc.vector.tensor_scalar(
                        out_sb[:, cpos * d : (cpos + 1) * d],
                        ps2[:, 0:d],
                        rec_t[:, cpos : cpos + 1],
                        0.0,
                        mybir.AluOpType.mult,
                        mybir.AluOpType.max,
                    )
                getattr(nc, outq).dma_start(
                    out[:, :].rearrange("(m c) dd -> m (c dd)", c=cg), out_sb[:]
                )

            for _rep in range(reps):
                emit_body(early=(_rep < early_reps))
    nc.compile()
    return nc


def make_host_inputs(x, adj_mat, U, n=N_NODES, d=D, shard=SHARD, n_cores=N_CORES):
    """Build per-core input maps from the full problem inputs (dtype/layout
    transforms only — all arithmetic on the data happens on-device)."""
    np8 = mybir.dt.np(mybir.dt.float8e4)
    njt2 = n // (2 * P)

    x = np.asarray(x, dtype=np.float32)
    U = np.asarray(U, dtype=np.float32)

    x_hi8 = x.astype(np8)
    x_lo8 = ((x - x_hi8.astype(np.float32)) * 16.0).astype(np8)
    xpp = np.zeros((n, P), np8)
    xpp[:, 0:d] = x_hi8
    xpp[:, d] = np.ones((), np.float32).astype(np8)
    xpp[:, d + 1 : 2 * d] = x_lo8[:, 0 : d - 1]
    xp_t = np.ascontiguousarray(
        xpp.reshape(njt2, 2, P, P).transpose(2, 0, 1, 3).reshape(P, njt2 * 2 * P)
    )

    ut_full = np.zeros((P, d + 1), np.float32)
    ut_full[0:d, 0:d] = U.T
    ut_full[d + 1 : 2 * d, 0:d] = U.T[0 : d - 1] / 16.0  # row d: 0 cancels deg
    ut_full[d, d] = 1.0  # deg-selector column -> PSUM col d holds deg_i

    adj8 = np.asarray(adj_mat).astype(np8)  # binary values: exact in fp8
    in_maps = []
    for g in range(n_cores):
        a_sh = np.ascontiguousarray(adj8[:, g * shard : (g + 1) * shard])
        in_maps.append({"a_sh": a_sh, "xp": xp_t, "ut": ut_full})
    return in_maps


_NC_CACHE = {}


def get_nc(**kw):
    key = repr(sorted(kw.items()))
    if key not in _NC_CACHE:
        _NC_CACHE[key] = build_nc(**kw)
    return _NC_CACHE[key]


def kernel(x, adj_mat, U, **run_kw):
    """Full inputs in, full output out. Shards across 8 NeuronCores."""
    in_maps = make_host_inputs(x, adj_mat, U)
    nc = get_nc()
    res = run_bass_kernel_spmd(nc, in_maps, core_ids=list(range(N_CORES)), **run_kw)
    out = np.concatenate(
        [np.asarray(res.results[g]["out_sh"]).astype(np.float32) for g in range(N_CORES)],
        axis=0,
    )
    if run_kw:
        kernel.last_result = res
    return out


# revision 6
# speedup vs baseline: 1.0178x; 1.0018x over previous
"""GNN message-passing layer on 8 Trainium2 NeuronCores.

Computes out[i] = relu(U @ (sum_{j: adj[j,i]>0} x[j]) / deg_i) where
deg_i = sum_j adj[j,i], for a dense binary adjacency matrix.

Sharding: columns i of adj (target nodes) are row-sharded across the 8
cores (core g owns i in [g*2048, (g+1)*2048)); x and U are replicated.

Each core streams its 32MB fp8 adjacency shard once through the tensor
engine (the binary adjacency is exact in fp8e4, so the host stages it as
raw fp8 bytes and the DMA is a plain HWDGE copy — no cast anywhere):

  stage 1:  one fp8 DoubleRow matmul pass. The PE contracts TWO 128-row
            j-tiles per instruction at 2 moving cols/cycle (4x the bf16
            rate). The full 128 stationary columns hold a hi/lo split of
            x so one pass gives ~fp16-grade accuracy:
              cols   0..63 : x_hi[d]     (fp8 of x, all 64 dims)
              col       64 : 1.0         (accumulates deg_i)
              cols 65..127 : x_lo[d]*16  (fp8 of (x - fp8(x))*16, d<63)
            PSUM accumulates [128, 2048] fp32 over all 64 tile-pairs.
  stage 2:  out[i,:] = relu((agg_hi @ U^T + agg_lo @ U^T/16) * (1/deg_i))
            via ONE 128-row-contraction matmul per 128-node chunk with
            strided stationary + the hi/lo recombine (and deg-row cancel)
            folded into the moving tensor [U^T | deg-sel | U^T[0:63]/16].
            A 65th moving column drops each chunk's deg into PSUM col 64
            already chunk-transposed, so the 1/deg scale comes from a
            [128,1] DVE reciprocal — no DRAM round-trip to re-layout deg.
            The relu/scale is fused in one DVE tensor_scalar per chunk;
            the output store is bf16 (halves store bytes; adds ~1.7e-3
            rel err, total ~3e-3, far under the 2e-2 gate) and the host
            casts back to f32.

DMA queue use (measured on hardware, slope-timed over NEFF-internal
repetition): the per-core HBM controller sustains only ~350 GB/s
steady-state no matter how many DGE queues are active, and a SINGLE
queue walking DRAM sequentially is the most efficient transport —
column- or region-splitting the stream across the two HWDGE rings
and/or SWDGE always degraded sustained throughput (queue arbitration
breaks row-buffer locality). Fat descriptors (4-32KB via row-grouped
SBUF layouts) also measured no better than the plain 2KB-per-partition
layout. So: the whole adjacency stream rides the sync/SP HWDGE ring
sequentially, while the small x''/U^T loads and the bf16 output store
ride the scalar/ACT ring where they perturb the bulk stream least.

Dim 63 of x has no lo half (only 127 stationary cols remain after deg);
its error is ~fp8-grade but affects 1/64 dims -> ~2.5e-3 overall rel
err, within the 2e-2 gate.
"""

import sys

sys.path.insert(0, "/opt/trn_rl_repo")

import numpy as np
from contextlib import ExitStack

import concourse.bass as bass
import concourse.bacc as bacc
import concourse.mybir as mybir
import concourse.tile as tile
from concourse.bass_utils import run_bass_kernel_spmd

N_CORES = 8
P = 128

N_NODES = 16384
D = 64
SHARD = N_NODES // N_CORES  # 2048

TPD2 = 8  # j-tile PAIRS per slab -> slab = [tpd2*2*128 rows, SHARD] (4MB)


def build_nc(
    n=N_NODES,
    d=D,
    shard=SHARD,
    tpd2=TPD2,
    reps=1,
    bufs=3,
    splits=None,
    xq="scalar",
    outq="scalar",
    out_bf16=True,
    const_bufs=1,
    early_reps=0,
    early_cols=(896, 1792),
):
    """reps > 1 repeats the whole kernel body inside one NEFF — used only to
    measure per-invocation time as a slope (the axon RPC overhead per launch
    is ~90ms, far above the kernel time)."""
    njt2 = n // (2 * P)
    nslab = njt2 // tpd2
    if splits is None:
        splits = (("sync", nslab),)
    cg = shard // P
    f32 = mybir.dt.float32
    f8 = mybir.dt.float8e4
    out_dt = mybir.dt.bfloat16 if out_bf16 else f32

    assert sum(ns for _, ns in splits) == nslab, splits

    # Bacc (not raw Bass): its compile() splits multi-semaphore waits into
    # event semaphores — TRN2 ISA instructions fit only one wait.
    nc = bacc.Bacc()
    a_sh = nc.declare_dram_parameter("a_sh", [n, shard], f8, isOutput=False)
    xp = nc.declare_dram_parameter("xp", [P, njt2 * 2 * P], f8, isOutput=False)
    ut = nc.declare_dram_parameter("ut", [P, d + 1], f32, isOutput=False)
    out = nc.declare_dram_parameter("out_sh", [shard, d], out_dt, isOutput=True)

    # issue/consume order: round-robin across queues, each walking its own
    # contiguous slab region (PSUM accumulation commutes, so the PE can
    # consume slabs in arrival order)
    order = []  # (queue, slab_idx)
    bounds = []
    acc = 0
    for q, ns in splits:
        bounds.append((q, acc, acc + ns))
        acc += ns
    cursors = [[q, s0, s1, s0] for q, s0, s1 in bounds]
    while True:
        progressed = False
        for cur in cursors:
            if cur[3] < cur[2]:
                order.append((cur[0], cur[3]))
                cur[3] += 1
                progressed = True
        if not progressed:
            break

    with tile.TileContext(nc) as tc:
        with ExitStack() as ctx:
            constp = ctx.enter_context(tc.tile_pool(name="const", bufs=const_bufs))
            utp = ctx.enter_context(tc.tile_pool(name="utp", bufs=1))
            apool = ctx.enter_context(tc.tile_pool(name="adj", bufs=bufs))
            pspool = ctx.enter_context(tc.tile_pool(name="psum1", bufs=1, space="PSUM"))
            ps2pool = ctx.enter_context(tc.tile_pool(name="psum2", bufs=2, space="PSUM"))
            postp = ctx.enter_context(tc.tile_pool(name="post", bufs=1))

            first_t2 = order[0][1] * tpd2
            last_t2 = order[-1][1] * tpd2 + (tpd2 - 1)

            def emit_body(early=False):
                # stationary: partition p holds x''[(2*t2+s)*128+p, c] at
                # free offset (t2, s, c); c = [x_hi | 1 | x_lo*16] layout.
                x_sb = constp.tile([P, njt2, 2, P], f8, tag="x")
                getattr(nc, xq).dma_start(
                    x_sb[:], xp[:, :].rearrange("p (t s c) -> p t s c", s=2, c=P)
                )
                ut_sb = utp.tile([P, d + 1], f32, tag="ut")
                getattr(nc, xq).dma_start(ut_sb[:], ut[:, :])

                # stage 1: aggT = x''^T @ a over all j-tile pairs (DoubleRow)
                ps_agg = pspool.tile([P, shard], f32, tag="agg")
                for q, sl in order:
                    rows = a_sh[sl * tpd2 * 2 * P : (sl + 1) * tpd2 * 2 * P, :]
                    a_t = apool.tile([P, tpd2, 2, shard], f8, tag="a")
                    src = rows.rearrange("(t s p) c -> p t s c", p=P, s=2)
                    if early:
                        # burst-phase transport: all three DGE queues split
                        # each slab's columns — the HBM controller exceeds
                        # 450 GB/s for the first ~2-3ms of an execution, and
                        # multi-queue is the only way to pull that rate.
                        b0, b1 = early_cols
                        nc.sync.dma_start(a_t[:, :, :, 0:b0], src[:, :, :, 0:b0])
                        nc.scalar.dma_start(
                            a_t[:, :, :, b0:b1], src[:, :, :, b0:b1]
                        )
                        nc.gpsimd.dma_start(a_t[:, :, :, b1:], src[:, :, :, b1:])
                    else:
                        getattr(nc, q).dma_start(a_t[:], src)
                    for tt in range(tpd2):
                        t2 = sl * tpd2 + tt
                        # 256-wide out chunks: the DoubleRow moving AP is
                        # [128, 2, w] = 2w free elements and the PE caps the
                        # moving free dim at 512 (wider hangs the engine).
                        # start=True zeroes the full 2KB PSUM bank, so only
                        # the bank-aligned chunk carries it; the odd chunk
                        # inherits the bank's pending-zero.
                        for c0 in range(0, shard, 256):
                            w = min(256, shard - c0)
                            nc.tensor.matmul(
                                ps_agg[:, c0 : c0 + w],
                                x_sb[:, t2, :, :],
                                a_t[:, tt, :, c0 : c0 + w],
                                start=(t2 == first_t2 and c0 % 512 == 0),
                                stop=(t2 == last_t2),
                                perf_mode=mybir.MatmulPerfMode.DoubleRow,
                                skip_group_check=(c0 % 512 != 0),
                            )

                # epilogue: rows 0..63 = agg_hi, row 64 = deg,
                # rows 65..127 = agg_lo*16.  Stays on DVE: ACT's sequencer
                # owns the scalar-ring aux DMAs.
                agg_sb = postp.tile([P, shard], f32, tag="aggsb")
                nc.vector.tensor_copy(agg_sb[:], ps_agg[:])

                out_sb = postp.tile([P, cg * d], out_dt, tag="out")
                rec_t = postp.tile([P, cg], f32, tag="rec")
                # i = m*cg + cpos: strided stationary picks every cg-th column
                agg_all = agg_sb[:, :].rearrange("dd (m c) -> dd c m", c=cg)
                for cpos in range(cg):
                    ps2 = ps2pool.tile([P, d + 1], f32, tag="p2")
                    nc.tensor.matmul(
                        ps2[:],
                        agg_all[:, cpos, :],
                        ut_sb[:, :],
                        start=True,
                        stop=True,
                    )
                    nc.vector.reciprocal(
                        rec_t[:, cpos : cpos + 1], ps2[:, d : d + 1]
                    )
                    # out = max(ps2 * (1/deg), 0) fused in one DVE op
                    nc.vector.tensor_scalar(
                        out_sb[:, cpos * d : (cpos + 1) * d],
                        ps2[:, 0:d],
                        rec_t[:, cpos : cpos + 1],
                        0.0,
                        mybir.AluOpType.mult,
                        mybir.AluOpType.max,
                    )
                getattr(nc, outq).dma_start(
                    out[:, :].rearrange("(m c) dd -> m (c dd)", c=cg), out_sb[:]
                )

            for _rep in range(reps):
                emit_body(early=(_rep < early_reps))
    nc.compile()
    return nc


def make_host_inputs(x, adj_mat, U, n=N_NODES, d=D, shard=SHARD, n_cores=N_CORES):
    """Build per-core input maps from the full problem inputs (dtype/layout
    transforms only — all arithmetic on the data happens on-device)."""
    np8 = mybir.dt.np(mybir.dt.float8e4)
    njt2 = n // (2 * P)

    x = np.asarray(x, dtype=np.float32)
    U = np.asarray(U, dtype=np.float32)

    x_hi8 = x.astype(np8)
    x_lo8 = ((x - x_hi8.astype(np.float32)) * 16.0).astype(np8)
    xpp = np.zeros((n, P), np8)
    xpp[:, 0:d] = x_hi8
    xpp[:, d] = np.ones((), np.float32).astype(np8)
    xpp[:, d + 1 : 2 * d] = x_lo8[:, 0 : d - 1]
    xp_t = np.ascontiguousarray(
        xpp.reshape(njt2, 2, P, P).transpose(2, 0, 1, 3).reshape(P, njt2 * 2 * P)
    )

    ut_full = np.zeros((P, d + 1), np.float32)
    ut_full[0:d, 0:d] = U.T
    ut_full[d + 1 : 2 * d, 0:d] = U.T[0 : d - 1] / 16.0  # row d: 0 cancels deg
    ut_full[d, d] = 1.0  # deg-selector column -> PSUM col d holds deg_i

    adj8 = np.asarray(adj_mat).astype(np8)  # binary values: exact in fp8
    in_maps = []
    for g in range(n_cores):
        a_sh = np.ascontiguousarray(adj8[:, g * shard : (g + 1) * shard])
        in_maps.append({"a_sh": a_sh, "xp": xp_t, "ut": ut_full})
    return in_maps


_NC_CACHE = {}


def get_nc(**kw):
    key = repr(sorted(kw.items()))
    if key not in _NC_CACHE:
        _NC_CACHE[key] = build_nc(**kw)
    return _NC_CACHE[key]


def kernel(x, adj_mat, U, **run_kw):
    """Full inputs in, full output out. Shards across 8 NeuronCores."""
    in_maps = make_host_inputs(x, adj_mat, U)
    nc = get_nc()
    res = run_bass_kernel_spmd(nc, in_maps, core_ids=list(range(N_CORES)), **run_kw)
    out = np.concatenate(
        [np.asarray(res.results[g]["out_sh"]).astype(np.float32) for g in range(N_CORES)],
        axis=0,
    )
    if run_kw:
        kernel.last_result = res
    return out


# revision 7
# speedup vs baseline: 1.0324x; 1.0143x over previous
"""GNN message-passing layer on 8 Trainium2 NeuronCores.

Computes out[i] = relu(U @ (sum_{j: adj[j,i]>0} x[j]) / deg_i) where
deg_i = sum_j adj[j,i], for a dense binary adjacency matrix.

Sharding: columns i of adj (target nodes) are row-sharded across the 8
cores (core g owns i in [g*2048, (g+1)*2048)); x and U are replicated.

Each core streams its 32MB fp8 adjacency shard once through the tensor
engine (the binary adjacency is exact in fp8e4, so the host stages it as
raw fp8 bytes and the DMA is a plain HWDGE copy — no cast anywhere):

  stage 1:  one fp8 DoubleRow matmul pass. The PE contracts TWO 128-row
            j-tiles per instruction at 2 moving cols/cycle (4x the bf16
            rate). The full 128 stationary columns hold a hi/lo split of
            x so one pass gives ~fp16-grade accuracy:
              cols   0..63 : x_hi[d]     (fp8 of x, all 64 dims)
              col       64 : 1.0         (accumulates deg_i)
              cols 65..127 : x_lo[d]*16  (fp8 of (x - fp8(x))*16, d<63)
            PSUM accumulates [128, 2048] fp32 over all 64 tile-pairs.
  stage 2:  out[i,:] = relu((agg_hi @ U^T + agg_lo @ U^T/16) * (1/deg_i))
            via ONE 128-row-contraction matmul per 128-node chunk with
            strided stationary + the hi/lo recombine (and deg-row cancel)
            folded into the moving tensor [U^T | deg-sel | U^T[0:63]/16].
            A 65th moving column drops each chunk's deg into PSUM col 64
            already chunk-transposed, so the 1/deg scale comes from a
            [128,1] DVE reciprocal — no DRAM round-trip to re-layout deg.
            The relu/scale is fused in one DVE tensor_scalar per chunk;
            the output store is bf16 (halves store bytes; adds ~1.7e-3
            rel err, total ~3e-3, far under the 2e-2 gate) and the host
            casts back to f32.

DMA queue use (measured on hardware, slope-timed over NEFF-internal
repetition): the per-core HBM controller sustains only ~350 GB/s
steady-state no matter how many DGE queues are active, and a SINGLE
queue walking DRAM sequentially is the most efficient transport —
column- or region-splitting the stream across the two HWDGE rings
and/or SWDGE always degraded sustained throughput (queue arbitration
breaks row-buffer locality). Fat descriptors (4-32KB via row-grouped
SBUF layouts) also measured no better than the plain 2KB-per-partition
layout. So: the whole adjacency stream rides the sync/SP HWDGE ring
sequentially, while the small x''/U^T loads and the bf16 output store
ride the scalar/ACT ring where they perturb the bulk stream least.

Dim 63 of x has no lo half (only 127 stationary cols remain after deg);
its error is ~fp8-grade but affects 1/64 dims -> ~2.5e-3 overall rel
err, within the 2e-2 gate.
"""

import sys

sys.path.insert(0, "/opt/trn_rl_repo")

import numpy as np
from contextlib import ExitStack

import concourse.bass as bass
import concourse.bacc as bacc
import concourse.mybir as mybir
import concourse.tile as tile
from concourse.bass_utils import run_bass_kernel_spmd

N_CORES = 8
P = 128

N_NODES = 16384
D = 64
SHARD = N_NODES // N_CORES  # 2048

TPD2 = 8  # j-tile PAIRS per slab -> slab = [tpd2*2*128 rows, SHARD] (4MB)


def build_nc(
    n=N_NODES,
    d=D,
    shard=SHARD,
    tpd2=TPD2,
    reps=1,
    bufs=4,
    splits=None,
    xq="scalar",
    outq="scalar",
    out_bf16=True,
    const_bufs=1,
    early_reps=0,
    early_cols=(896, 1792),
):
    """reps > 1 repeats the whole kernel body inside one NEFF — used only to
    measure per-invocation time as a slope (the axon RPC overhead per launch
    is ~90ms, far above the kernel time)."""
    njt2 = n // (2 * P)
    nslab = njt2 // tpd2
    if splits is None:
        splits = (("sync", nslab),)
    cg = shard // P
    f32 = mybir.dt.float32
    f8 = mybir.dt.float8e4
    out_dt = mybir.dt.bfloat16 if out_bf16 else f32

    assert sum(ns for _, ns in splits) == nslab, splits

    # Bacc (not raw Bass): its compile() splits multi-semaphore waits into
    # event semaphores — TRN2 ISA instructions fit only one wait.
    nc = bacc.Bacc()
    a_sh = nc.declare_dram_parameter("a_sh", [n, shard], f8, isOutput=False)
    xp = nc.declare_dram_parameter("xp", [P, njt2 * 2 * P], f8, isOutput=False)
    ut = nc.declare_dram_parameter("ut", [P, d + 1], f32, isOutput=False)
    out = nc.declare_dram_parameter("out_sh", [shard, d], out_dt, isOutput=True)

    # issue/consume order: round-robin across queues, each walking its own
    # contiguous slab region (PSUM accumulation commutes, so the PE can
    # consume slabs in arrival order)
    order = []  # (queue, slab_idx)
    bounds = []
    acc = 0
    for q, ns in splits:
        bounds.append((q, acc, acc + ns))
        acc += ns
    cursors = [[q, s0, s1, s0] for q, s0, s1 in bounds]
    while True:
        progressed = False
        for cur in cursors:
            if cur[3] < cur[2]:
                order.append((cur[0], cur[3]))
                cur[3] += 1
                progressed = True
        if not progressed:
            break

    with tile.TileContext(nc) as tc:
        with ExitStack() as ctx:
            constp = ctx.enter_context(tc.tile_pool(name="const", bufs=const_bufs))
            utp = ctx.enter_context(tc.tile_pool(name="utp", bufs=1))
            apool = ctx.enter_context(tc.tile_pool(name="adj", bufs=bufs))
            pspool = ctx.enter_context(tc.tile_pool(name="psum1", bufs=1, space="PSUM"))
            ps2pool = ctx.enter_context(tc.tile_pool(name="psum2", bufs=2, space="PSUM"))
            postp = ctx.enter_context(tc.tile_pool(name="post", bufs=1))

            first_t2 = order[0][1] * tpd2
            last_t2 = order[-1][1] * tpd2 + (tpd2 - 1)

            def emit_body(early=False):
                # stationary: partition p holds x''[(2*t2+s)*128+p, c] at
                # free offset (t2, s, c); c = [x_hi | 1 | x_lo*16] layout.
                x_sb = constp.tile([P, njt2, 2, P], f8, tag="x")
                getattr(nc, xq).dma_start(
                    x_sb[:], xp[:, :].rearrange("p (t s c) -> p t s c", s=2, c=P)
                )
                ut_sb = utp.tile([P, d + 1], f32, tag="ut")
                getattr(nc, xq).dma_start(ut_sb[:], ut[:, :])

                # stage 1: aggT = x''^T @ a over all j-tile pairs (DoubleRow)
                ps_agg = pspool.tile([P, shard], f32, tag="agg")
                for q, sl in order:
                    rows = a_sh[sl * tpd2 * 2 * P : (sl + 1) * tpd2 * 2 * P, :]
                    a_t = apool.tile([P, tpd2, 2, shard], f8, tag="a")
                    src = rows.rearrange("(t s p) c -> p t s c", p=P, s=2)
                    if early:
                        # burst-phase transport: all three DGE queues split
                        # each slab's columns — the HBM controller exceeds
                        # 450 GB/s for the first ~2-3ms of an execution, and
                        # multi-queue is the only way to pull that rate.
                        b0, b1 = early_cols
                        nc.sync.dma_start(a_t[:, :, :, 0:b0], src[:, :, :, 0:b0])
                        nc.scalar.dma_start(
                            a_t[:, :, :, b0:b1], src[:, :, :, b0:b1]
                        )
                        nc.gpsimd.dma_start(a_t[:, :, :, b1:], src[:, :, :, b1:])
                    else:
                        getattr(nc, q).dma_start(a_t[:], src)
                    for tt in range(tpd2):
                        t2 = sl * tpd2 + tt
                        # 256-wide out chunks: the DoubleRow moving AP is
                        # [128, 2, w] = 2w free elements and the PE caps the
                        # moving free dim at 512 (wider hangs the engine).
                        # start=True zeroes the full 2KB PSUM bank, so only
                        # the bank-aligned chunk carries it; the odd chunk
                        # inherits the bank's pending-zero.
                        for c0 in range(0, shard, 256):
                            w = min(256, shard - c0)
                            nc.tensor.matmul(
                                ps_agg[:, c0 : c0 + w],
                                x_sb[:, t2, :, :],
                                a_t[:, tt, :, c0 : c0 + w],
                                start=(t2 == first_t2 and c0 % 512 == 0),
                                stop=(t2 == last_t2),
                                perf_mode=mybir.MatmulPerfMode.DoubleRow,
                                skip_group_check=(c0 % 512 != 0),
                            )

                # epilogue: rows 0..63 = agg_hi, row 64 = deg,
                # rows 65..127 = agg_lo*16.  Stays on DVE: ACT's sequencer
                # owns the scalar-ring aux DMAs.
                agg_sb = postp.tile([P, shard], f32, tag="aggsb")
                nc.vector.tensor_copy(agg_sb[:], ps_agg[:])

                out_sb = postp.tile([P, cg * d], out_dt, tag="out")
                rec_t = postp.tile([P, cg], f32, tag="rec")
                # i = m*cg + cpos: strided stationary picks every cg-th column
                agg_all = agg_sb[:, :].rearrange("dd (m c) -> dd c m", c=cg)
                for cpos in range(cg):
                    ps2 = ps2pool.tile([P, d + 1], f32, tag="p2")
                    nc.tensor.matmul(
                        ps2[:],
                        agg_all[:, cpos, :],
                        ut_sb[:, :],
                        start=True,
                        stop=True,
                    )
                    nc.vector.reciprocal(
                        rec_t[:, cpos : cpos + 1], ps2[:, d : d + 1]
                    )
                    # out = max(ps2 * (1/deg), 0) fused in one DVE op
                    nc.vector.tensor_scalar(
                        out_sb[:, cpos * d : (cpos + 1) * d],
                        ps2[:, 0:d],
                        rec_t[:, cpos : cpos + 1],
                        0.0,
                        mybir.AluOpType.mult,
                        mybir.AluOpType.max,
                    )
                getattr(nc, outq).dma_start(
                    out[:, :].rearrange("(m c) dd -> m (c dd)", c=cg), out_sb[:]
                )

            for _rep in range(reps):
                emit_body(early=(_rep < early_reps))
    nc.compile()
    return nc


def make_host_inputs(x, adj_mat, U, n=N_NODES, d=D, shard=SHARD, n_cores=N_CORES):
    """Build per-core input maps from the full problem inputs (dtype/layout
    transforms only — all arithmetic on the data happens on-device)."""
    np8 = mybir.dt.np(mybir.dt.float8e4)
    njt2 = n // (2 * P)

    x = np.asarray(x, dtype=np.float32)
    U = np.asarray(U, dtype=np.float32)

    x_hi8 = x.astype(np8)
    x_lo8 = ((x - x_hi8.astype(np.float32)) * 16.0).astype(np8)
    xpp = np.zeros((n, P), np8)
    xpp[:, 0:d] = x_hi8
    xpp[:, d] = np.ones((), np.float32).astype(np8)
    xpp[:, d + 1 : 2 * d] = x_lo8[:, 0 : d - 1]
    xp_t = np.ascontiguousarray(
        xpp.reshape(njt2, 2, P, P).transpose(2, 0, 1, 3).reshape(P, njt2 * 2 * P)
    )

    ut_full = np.zeros((P, d + 1), np.float32)
    ut_full[0:d, 0:d] = U.T
    ut_full[d + 1 : 2 * d, 0:d] = U.T[0 : d - 1] / 16.0  # row d: 0 cancels deg
    ut_full[d, d] = 1.0  # deg-selector column -> PSUM col d holds deg_i

    adj8 = np.asarray(adj_mat).astype(np8)  # binary values: exact in fp8
    in_maps = []
    for g in range(n_cores):
        a_sh = np.ascontiguousarray(adj8[:, g * shard : (g + 1) * shard])
        in_maps.append({"a_sh": a_sh, "xp": xp_t, "ut": ut_full})
    return in_maps


_NC_CACHE = {}


def get_nc(**kw):
    key = repr(sorted(kw.items()))
    if key not in _NC_CACHE:
        _NC_CACHE[key] = build_nc(**kw)
    return _NC_CACHE[key]


def kernel(x, adj_mat, U, **run_kw):
    """Full inputs in, full output out. Shards across 8 NeuronCores."""
    in_maps = make_host_inputs(x, adj_mat, U)
    nc = get_nc()
    res = run_bass_kernel_spmd(nc, in_maps, core_ids=list(range(N_CORES)), **run_kw)
    out = np.concatenate(
        [np.asarray(res.results[g]["out_sh"]).astype(np.float32) for g in range(N_CORES)],
        axis=0,
    )
    if run_kw:
        kernel.last_result = res
    return out
